# revision 1
# baseline (speedup 1.0000x reference)
"""Trainium2 Bass kernel for nn_CopyMechanismMixin (copy-mechanism + vocab projection).

Sharding: vocab-parallel across 8 cores for the dominant [1024tok,1024]x[1024,50257]
projection + softmax + scatter; token-parallel (1 batch x 128 tokens per core) for
the small copy-attention prologue. Cross-core: one AllGather (copy probs, bf16) and
one AllReduce (softmax denominators, f32). Scatter-add of copy probs into the vocab
distribution is done with indirect-DMA row gather/scatter on the output table in
[vocab, token] layout, after merging duplicate vocab ids with is_equal selection
matmuls.
"""

import numpy as np
import ml_dtypes

import concourse.bass as bass
import concourse.bacc as bacc
import concourse.mybir as mybir
import concourse.tile as tile
from concourse.bass_utils import run_bass_kernel_spmd
from concourse.masks import make_identity

F32 = mybir.dt.float32
BF16 = mybir.dt.bfloat16
I32 = mybir.dt.int32
BF = ml_dtypes.bfloat16
AF = mybir.ActivationFunctionType
ALU = mybir.AluOpType

B, T, M, D, V = 4, 256, 512, 1024, 50257
NCORES = 8
VS = -(-V // NCORES)          # 6283 per-core vocab shard
NVT = -(-VS // 128)           # 50 vocab tiles per core
VP = NVT * 128                # 6400 padded shard width
SENT = VP - 1                 # sentinel row (pad region)
PAD_BIAS = -30.0              # b_out value for pad rows -> exp ~ 1e-13
MASK_NEG = -30000.0           # additive score mask
NT = B * T                    # 1024 total tokens
TC = 128                      # tokens per core (attention phase)
KD = D // 128                 # 8 contraction chunks of 128
KE = 2 * D // 128             # 16


def build_kernel():
    nc = bacc.Bacc(
        "TRN2",
        target_bir_lowering=False,
        debug=False,
        enable_asserts=False,
        num_devices=NCORES,
    )
    # ---- I/O ----
    dec_myT = nc.dram_tensor("dec_myT", [D, TC], F32, kind="ExternalInput")
    dec_myT_bf = nc.dram_tensor("dec_myT_bf", [D, TC], BF16, kind="ExternalInput")
    decT = nc.dram_tensor("decT", [KD, 128, NT], BF16, kind="ExternalInput")
    wcopy = nc.dram_tensor("wcopy", [D, D], F32, kind="ExternalInput")
    wdecT = nc.dram_tensor("wdecT", [2 * D, D], BF16, kind="ExternalInput")
    wgenT = nc.dram_tensor("wgenT", [D, 1], BF16, kind="ExternalInput")
    bdec = nc.dram_tensor("bdec", [D, 1], F32, kind="ExternalInput")
    bgen = nc.dram_tensor("bgen", [128, 1], F32, kind="ExternalInput")
    membT = nc.dram_tensor("membT", [D, M], F32, kind="ExternalInput")
    memb = nc.dram_tensor("memb", [M, D], F32, kind="ExternalInput")
    maskb = nc.dram_tensor("maskb", [TC, M], F32, kind="ExternalInput")
    woutT = nc.dram_tensor("woutT", [NVT, KD, 128, 128], BF16, kind="ExternalInput")
    bo = nc.dram_tensor("bo", [128, NVT], F32, kind="ExternalInput")
    ids_f = nc.dram_tensor("ids_f", [B, 4, 128, 1], F32, kind="ExternalInput")
    ids_loc = nc.dram_tensor("ids_loc", [B, 4, 128, 1], I32, kind="ExternalInput")
    outb = [
        nc.dram_tensor(f"out{b}", [VP, T], F32, kind="ExternalOutput") for b in range(B)
    ]

    with tile.TileContext(nc) as tc:
        with (
            tc.tile_pool(name="const", bufs=1) as constp,
            tc.tile_pool(name="wstream", bufs=4) as wsp,
            tc.tile_pool(name="attn", bufs=1) as ap,
            tc.tile_pool(name="small", bufs=2) as sp,
            tc.tile_pool(name="stash", bufs=54) as stashp,
            tc.tile_pool(name="wout", bufs=3) as wop,
            tc.tile_pool(name="fin", bufs=3) as finp,
            tc.tile_pool(name="mrg", bufs=2) as mrgp,
            tc.tile_pool(name="psA", bufs=2, space="PSUM") as psA,
            tc.tile_pool(name="psB", bufs=2, space="PSUM") as psB,
            tc.tile_pool(name="psS", bufs=2, space="PSUM") as psS,
            tc.tile_pool(name="dram", bufs=1, space="DRAM") as dram,
        ):
            ident = constp.tile([128, 128], F32, tag="ident")
            make_identity(nc, ident[:])

            # ============ Phase A: copy-attention for my 128 tokens ============
            # dec_myT (f32) -> SBUF [128, KD*128]
            dmt = constp.tile([128, KD * 128], F32, tag="dmt")
            for kd in range(KD):
                nc.sync.dma_start(
                    out=dmt[:, kd * 128 : (kd + 1) * 128],
                    in_=dec_myT[kd * 128 : (kd + 1) * 128, :],
                )
            dmt_bf = constp.tile([128, KD * 128], BF16, tag="dmtbf")
            for kd in range(KD):
                nc.sync.dma_start(
                    out=dmt_bf[:, kd * 128 : (kd + 1) * 128],
                    in_=dec_myT_bf[kd * 128 : (kd + 1) * 128, :],
                )

            # dprojT[d, t] = sum_e W_copy[e, d] * decT[e, t]
            dpT = constp.tile([128, KD * 128], F32, tag="dpT")
            for dc in range(KD):
                ps = psA.tile([128, 128], F32, space="PSUM", tag="psa")
                for ke in range(KD):
                    wct = wsp.tile([128, 128], F32, tag="wc")
                    nc.sync.dma_start(
                        out=wct[:],
                        in_=wcopy[
                            ke * 128 : (ke + 1) * 128, dc * 128 : (dc + 1) * 128
                        ],
                    )
                    nc.tensor.matmul(
                        out=ps[:],
                        lhsT=wct[:],
                        rhs=dmt[:, ke * 128 : (ke + 1) * 128],
                        start=(ke == 0),
                        stop=(ke == KD - 1),
                    )
                nc.scalar.copy(dpT[:, dc * 128 : (dc + 1) * 128], ps[:])

            # scores[t, m] = sum_d dprojT[d, t] * memT[d, m]
            scps = psB.tile([128, M], F32, space="PSUM", tag="psb")
            for dc in range(KD):
                mTt = wsp.tile([128, M], F32, tag="mT")
                nc.sync.dma_start(
                    out=mTt[:], in_=membT[dc * 128 : (dc + 1) * 128, :]
                )
                nc.tensor.matmul(
                    out=scps[:],
                    lhsT=dpT[:, dc * 128 : (dc + 1) * 128],
                    rhs=mTt[:],
                    start=(dc == 0),
                    stop=(dc == KD - 1),
                )
            mbt = ap.tile([128, M], F32, tag="mbt")
            nc.sync.dma_start(out=mbt[:], in_=maskb[:])
            sc = ap.tile([128, M], F32, tag="sc")
            nc.vector.tensor_tensor(out=sc[:], in0=scps[:], in1=mbt[:], op=ALU.add)
            mx = sp.tile([128, 1], F32, tag="mx")
            nc.vector.reduce_max(out=mx[:], in_=sc[:], axis=mybir.AxisListType.X)
            nmx = sp.tile([128, 1], F32, tag="nmx")
            nc.vector.tensor_scalar_mul(nmx[:], mx[:], -1.0)
            esc = ap.tile([128, M], F32, tag="esc")
            sesum = sp.tile([128, 1], F32, tag="sesum")
            nc.scalar.activation(out=esc[:], in_=sc[:], func=AF.Exp, bias=nmx[:, :1])
            nc.vector.reduce_sum(out=sesum[:], in_=esc[:], axis=mybir.AxisListType.X)
            rinv = sp.tile([128, 1], F32, tag="rinv")
            nc.vector.reciprocal(rinv[:], sesum[:])
            attn = ap.tile([128, M], F32, tag="attn")
            nc.vector.tensor_scalar_mul(attn[:], esc[:], rinv[:, :1])

            # attnT via PE transpose -> [m-part x4, 128 t] f32
            aT = ap.tile([128, 4 * 128], F32, tag="aT")
            for mc in range(4):
                tp = psA.tile([128, 128], F32, space="PSUM", tag="psa")
                nc.tensor.transpose(
                    out=tp[:], in_=attn[:, mc * 128 : (mc + 1) * 128], identity=ident[:]
                )
                nc.scalar.copy(aT[:, mc * 128 : (mc + 1) * 128], tp[:])

            # attn_outT[d, t] = sum_m memb[m, d] * attnT[m, t]  -> bf16
            aoT_bf = ap.tile([128, KD * 128], BF16, tag="aoT")
            for dc in range(KD):
                ps = psA.tile([128, 128], F32, space="PSUM", tag="psa")
                for mc in range(4):
                    mbt2 = wsp.tile([128, 128], F32, tag="memb")
                    nc.sync.dma_start(
                        out=mbt2[:],
                        in_=memb[
                            mc * 128 : (mc + 1) * 128, dc * 128 : (dc + 1) * 128
                        ],
                    )
                    nc.tensor.matmul(
                        out=ps[:],
                        lhsT=mbt2[:],
                        rhs=aT[:, mc * 128 : (mc + 1) * 128],
                        start=(mc == 0),
                        stop=(mc == 3),
                    )
                nc.scalar.copy(aoT_bf[:, dc * 128 : (dc + 1) * 128], ps[:])

            # dwaT[d, t] = tanh(sum_e W_decT[e, d] * dec_catT[e, t] + b_dec[d]) bf16
            bd = constp.tile([128, KD], F32, tag="bd")
            nc.sync.dma_start(
                out=bd[:], in_=bdec[:].rearrange("(a p) o -> p (a o)", p=128)
            )
            th = ap.tile([128, KD * 128], BF16, tag="th")
            for dc in range(KD):
                ps = psA.tile([128, 128], F32, space="PSUM", tag="psa")
                for ec in range(KE):
                    wdt = wsp.tile([128, 128], BF16, tag="wd")
                    nc.sync.dma_start(
                        out=wdt[:],
                        in_=wdecT[
                            ec * 128 : (ec + 1) * 128, dc * 128 : (dc + 1) * 128
                        ],
                    )
                    rhs = (
                        dmt_bf[:, ec * 128 : (ec + 1) * 128]
                        if ec < KD
                        else aoT_bf[:, (ec - KD) * 128 : (ec - KD + 1) * 128]
                    )
                    nc.tensor.matmul(
                        out=ps[:], lhsT=wdt[:], rhs=rhs, start=(ec == 0), stop=(ec == KE - 1)
                    )
                nc.scalar.activation(
                    out=th[:, dc * 128 : (dc + 1) * 128],
                    in_=ps[:],
                    func=AF.Tanh,
                    bias=bd[:, dc : dc + 1],
                )

            # z[t] = sum_d W_gen[d] * dwaT[d, t] ; pg = sigmoid(z + b_gen)
            wg = constp.tile([128, KD], BF16, tag="wg")
            nc.sync.dma_start(
                out=wg[:], in_=wgenT[:].rearrange("(a p) o -> p (a o)", p=128)
            )
            zps = psA.tile([128, 128], F32, space="PSUM", tag="psa")
            for dc in range(KD):
                nc.tensor.matmul(
                    out=zps[:, :1],
                    lhsT=th[:, dc * 128 : (dc + 1) * 128],
                    rhs=wg[:, dc : dc + 1],
                    start=(dc == 0),
                    stop=(dc == KD - 1),
                )
            bg = constp.tile([128, 1], F32, tag="bg")
            nc.sync.dma_start(out=bg[:], in_=bgen[:])
            pg = sp.tile([128, 1], F32, tag="pg")
            nc.scalar.activation(
                out=pg[:], in_=zps[:, :1], func=AF.Sigmoid, bias=bg[:, :1]
            )
            # cp = esc * (rinv * (1 - pg))   (f32), then transpose+cast to bf16
            ompg = sp.tile([128, 1], F32, tag="ompg")
            nc.vector.tensor_scalar(
                out=ompg[:], in0=pg[:], scalar1=-1.0, scalar2=1.0, op0=ALU.mult, op1=ALU.add
            )
            s2 = sp.tile([128, 1], F32, tag="s2")
            nc.vector.tensor_tensor(out=s2[:], in0=rinv[:], in1=ompg[:], op=ALU.mult)
            cp = ap.tile([128, M], F32, tag="cp")
            nc.vector.tensor_scalar_mul(cp[:], esc[:], s2[:, :1])

            # AG contribution [M+1, 128] bf16: rows 0..511 cpT, row 512 pg
            ag_in = dram.tile([M + 1, TC], BF16)
            cpT_bf = ap.tile([128, 128], BF16, tag="cpTbf")
            for mc in range(4):
                tp = psA.tile([128, 128], F32, space="PSUM", tag="psa")
                nc.tensor.transpose(
                    out=tp[:], in_=cp[:, mc * 128 : (mc + 1) * 128], identity=ident[:]
                )
                nc.scalar.copy(cpT_bf[:], tp[:])
                nc.sync.dma_start(
                    out=ag_in[mc * 128 : (mc + 1) * 128, :], in_=cpT_bf[:]
                )
            pgpad = ap.tile([128, 128], F32, tag="pgpad")
            nc.vector.memset(pgpad[:], 0.0)
            nc.vector.tensor_copy(out=pgpad[:, 0:1], in_=pg[:])
            tp = psA.tile([128, 128], F32, space="PSUM", tag="psa")
            nc.tensor.transpose(out=tp[:], in_=pgpad[:], identity=ident[:])
            pgT_bf = sp.tile([1, 128], BF16, tag="pgT")
            nc.scalar.copy(pgT_bf[:], tp[0:1, :])
            nc.sync.dma_start(out=ag_in[M : M + 1, :], in_=pgT_bf[:])

            ag_out = dram.tile([NCORES * (M + 1), TC], BF16, addr_space="Shared")
            nc.gpsimd.collective_compute(
                "AllGather",
                ALU.bypass,
                replica_groups=[list(range(NCORES))],
                ins=[ag_in[:].opt()],
                outs=[ag_out[:].opt()],
            )

            # ===== Phase B: vocab-shard logits/softmax, 2 token rounds =====
            bos = constp.tile([128, NVT], F32, tag="bos")
            nc.sync.dma_start(out=bos[:], in_=bo[:])
            ones_bf = constp.tile([128, 1], BF16, tag="ones_bf")
            nc.vector.memset(ones_bf[:], 1.0)
            ones128 = constp.tile([128, 128], F32, tag="ones128")
            nc.vector.memset(ones128[:], 1.0)

            # pg row (all tokens) + cpT_all [m-part x4, NT] from AG output
            pgrow_bf = sp.tile([1, NT], BF16, tag="pgrowbf", bufs=1)
            for c in range(NCORES):
                nc.sync.dma_start(
                    out=pgrow_bf[:, c * TC : (c + 1) * TC],
                    in_=ag_out[c * (M + 1) + M : c * (M + 1) + M + 1, :],
                )
            pgrow = sp.tile([1, NT], F32, tag="pgrow", bufs=1)
            nc.vector.tensor_copy(out=pgrow[:], in_=pgrow_bf[:])
            cpT = constp.tile([128, 4 * NT], BF16, tag="cpT")
            for c in range(NCORES):
                for mc in range(4):
                    nc.sync.dma_start(
                        out=cpT[:, mc * NT + c * TC : mc * NT + (c + 1) * TC],
                        in_=ag_out[
                            c * (M + 1) + mc * 128 : c * (M + 1) + (mc + 1) * 128, :
                        ],
                    )

            for r in range(2):
                tok0 = r * 512
                da = wop.tile([128, KD * 512], BF16, tag="da", bufs=2, name=f"da{r}")
                for kd in range(KD):
                    nc.sync.dma_start(
                        out=da[:, kd * 512 : (kd + 1) * 512],
                        in_=decT[kd][:, tok0 : tok0 + 512],
                    )
                s_acc = sp.tile([1, 512], F32, tag="sacc", name=f"sacc{r}")
                nc.vector.memset(s_acc[:], 0.0)
                stash = []
                for vt in range(NVT):
                    wot = wop.tile(
                        [128, KD * 128], BF16, tag="wot", bufs=3, name=f"wot{r}_{vt}"
                    )
                    for kd in range(KD):
                        nc.sync.dma_start(
                            out=wot[:, kd * 128 : (kd + 1) * 128], in_=woutT[vt, kd]
                        )
                    st = stashp.tile(
                        [128, 512], BF16, tag="stash", name=f"st{r}_{vt}"
                    )
                    ps = psB.tile([128, 512], F32, space="PSUM", tag="psb", name="pslg")
                    for kd in range(KD):
                        nc.tensor.matmul(
                            out=ps[:],
                            lhsT=wot[:, kd * 128 : (kd + 1) * 128],
                            rhs=da[:, kd * 512 : (kd + 1) * 512],
                            start=(kd == 0),
                            stop=(kd == KD - 1),
                        )
                    nc.scalar.activation(
                        out=st[:], in_=ps[:], func=AF.Exp, bias=bos[:, vt : vt + 1]
                    )
                    spp = psS.tile(
                        [1, 512], F32, space="PSUM", tag="psS", name=f"spp{r}_{vt}"
                    )
                    nc.tensor.matmul(
                        out=spp[:], lhsT=ones_bf[:], rhs=st[:], start=True, stop=True
                    )
                    nc.vector.tensor_tensor(
                        out=s_acc[:], in0=s_acc[:], in1=spp[:], op=ALU.add
                    )
                    stash.append(st)

                ar_in = dram.tile([1, 512], F32, tag=f"ar_in{r}", name=f"ar_in{r}")
                ar_out = dram.tile(
                    [1, 512], F32, addr_space="Shared", tag=f"ar_out{r}", name=f"ar_out{r}"
                )
                nc.sync.dma_start(out=ar_in[:], in_=s_acc[:])
                nc.gpsimd.collective_compute(
                    "AllReduce",
                    ALU.add,
                    replica_groups=[list(range(NCORES))],
                    ins=[ar_in[:].opt()],
                    outs=[ar_out[:].opt()],
                )
                s_glob = sp.tile([1, 512], F32, tag="sglob", name=f"sglob{r}")
                nc.sync.dma_start(out=s_glob[:], in_=ar_out[:])
                sinv = sp.tile([1, 512], F32, tag="sinv", name=f"sinv{r}")
                nc.vector.reciprocal(sinv[:], s_glob[:])
                crow = sp.tile([1, 512], F32, tag="crow", name=f"crow{r}")
                nc.vector.tensor_tensor(
                    out=crow[:],
                    in0=pgrow[:, tok0 : tok0 + 512],
                    in1=sinv[:],
                    op=ALU.mult,
                )
                zc = finp.tile([128, 512], F32, tag="x", name=f"zc{r}")
                nc.vector.memset(zc[:], 0.0)
                nc.vector.tensor_copy(out=zc[0:1, :], in_=crow[:])
                psC = psB.tile([128, 512], F32, space="PSUM", tag="psb", name=f"psC{r}")
                nc.tensor.matmul(
                    out=psC[:], lhsT=ones128[:], rhs=zc[:], start=True, stop=True
                )
                Cbf = wop.tile([128, 512], BF16, tag="Cbf", bufs=2, name=f"Cbf{r}")
                nc.scalar.copy(Cbf[:], psC[:])

                for vt in range(NVT):
                    x = finp.tile([128, 512], F32, tag="x", name=f"x{r}_{vt}")
                    nc.vector.tensor_tensor(
                        out=x[:], in0=stash[vt][:], in1=Cbf[:], op=ALU.mult
                    )
                    y = finp.tile([128, 512], F32, tag="y", name=f"y{r}_{vt}")
                    nc.scalar.activation(out=y[:], in_=x[:], func=AF.Ln)
                    for bb in range(2):
                        nc.sync.dma_start(
                            out=outb[2 * r + bb][vt * 128 : (vt + 1) * 128, :],
                            in_=y[:, bb * T : (bb + 1) * T],
                        )

                # ===== scatter fixup for this round's two batches =====
                for bb in range(2):
                    b = 2 * r + bb
                    idf = mrgp.tile([128, 4], F32, tag="idf", name=f"idf{b}")
                    for mc in range(4):
                        nc.sync.dma_start(out=idf[:, mc : mc + 1], in_=ids_f[b, mc])
                    idT = mrgp.tile([128, 4 * 128], F32, tag="idT", name=f"idT{b}")
                    for mc in range(4):
                        tp = psA.tile(
                            [128, 128], F32, space="PSUM", tag="psa", name=f"tpi{b}_{mc}"
                        )
                        nc.tensor.transpose(
                            out=tp[:],
                            in_=idf[:, mc : mc + 1].to_broadcast([128, 128]),
                            identity=ident[:],
                        )
                        nc.scalar.copy(idT[:, mc * 128 : (mc + 1) * 128], tp[:])
                    mg = mrgp.tile([128, 4 * T], F32, tag="mg", name=f"mg{b}")
                    for mi in range(4):
                        ps = psA.tile(
                            [128, T], F32, space="PSUM", tag="psa", name=f"psm{b}_{mi}"
                        )
                        for mj in range(4):
                            sel = mrgp.tile(
                                [128, 128], BF16, tag="sel", name=f"sel{b}_{mi}_{mj}"
                            )
                            nc.vector.tensor_tensor(
                                out=sel[:],
                                in0=idf[:, mj : mj + 1].to_broadcast([128, 128]),
                                in1=idT[:, mi * 128 : (mi + 1) * 128],
                                op=ALU.is_equal,
                            )
                            nc.tensor.matmul(
                                out=ps[:],
                                lhsT=sel[:],
                                rhs=cpT[:, mj * NT + b * T : mj * NT + (b + 1) * T],
                                start=(mj == 0),
                                stop=(mj == 3),
                            )
                        nc.scalar.copy(mg[:, mi * T : (mi + 1) * T], ps[:])

                    for mc in range(4):
                        ilc = mrgp.tile([128, 1], I32, tag="ilc", name=f"ilc{b}_{mc}")
                        nc.sync.dma_start(out=ilc[:], in_=ids_loc[b, mc])
                        g = mrgp.tile([128, T], F32, tag="g", name=f"g{b}_{mc}")
                        nc.gpsimd.indirect_dma_start(
                            out=g[:],
                            out_offset=None,
                            in_=outb[b][:],
                            in_offset=bass.IndirectOffsetOnAxis(ap=ilc[:, :1], axis=0),
                        )
                        gx = mrgp.tile([128, T], F32, tag="gx", name=f"gx{b}_{mc}")
                        nc.scalar.activation(out=gx[:], in_=g[:], func=AF.Exp)
                        nc.vector.tensor_tensor(
                            out=gx[:],
                            in0=gx[:],
                            in1=mg[:, mc * T : (mc + 1) * T],
                            op=ALU.add,
                        )
                        gz = mrgp.tile([128, T], F32, tag="gz", name=f"gz{b}_{mc}")
                        nc.scalar.activation(out=gz[:], in_=gx[:], func=AF.Ln)
                        nc.gpsimd.indirect_dma_start(
                            out=outb[b][:],
                            out_offset=bass.IndirectOffsetOnAxis(ap=ilc[:, :1], axis=0),
                            in_=gz[:],
                            in_offset=None,
                        )
    nc.finalize()
    return nc


_NC_CACHE = {}


def _get_nc():
    if "nc" not in _NC_CACHE:
        _NC_CACHE["nc"] = build_kernel()
    return _NC_CACHE["nc"]


def kernel(
    decoder_output,
    memory_output,
    memory_sequence_length,
    memory_ids,
    W_copy,
    b_copy,
    W_dec,
    b_dec,
    W_gen,
    b_gen,
    W_out,
    b_out,
):
    decoder_output = np.asarray(decoder_output, dtype=np.float32)
    memory_output = np.asarray(memory_output, dtype=np.float32)
    msl = np.asarray(memory_sequence_length).astype(np.int64)
    ids = np.asarray(memory_ids).astype(np.int64)
    W_copy = np.asarray(W_copy, dtype=np.float32)
    W_dec = np.asarray(W_dec, dtype=np.float32)
    W_gen = np.asarray(W_gen, dtype=np.float32)
    b_dec_a = np.asarray(b_dec, dtype=np.float32)
    b_gen_a = np.asarray(b_gen, dtype=np.float32)
    W_out = np.asarray(W_out, dtype=np.float32)
    b_out_a = np.asarray(b_out, dtype=np.float32)
    # NOTE: b_copy drops out: it shifts scores by a per-token constant, which
    # softmax over the memory axis cancels exactly.

    # ---- shared (core-independent) host prep ----
    dec_flat = decoder_output.reshape(NT, D)  # token g = b*T + t
    decT_all = np.ascontiguousarray(dec_flat.T)  # [D, NT]
    decT_bf = np.ascontiguousarray(
        decT_all.reshape(KD, 128, NT).astype(BF)
    )  # [KD,128,NT]
    wdecT = np.ascontiguousarray(W_dec.T.astype(BF))  # [2D, D]
    wgenT = np.ascontiguousarray(W_gen.reshape(1, D).T.astype(BF))  # [D,1]
    bdec_h = np.ascontiguousarray(b_dec_a.reshape(D, 1))
    bgen_h = np.full((128, 1), float(b_gen_a.ravel()[0]), np.float32)
    ids_f_h = np.ascontiguousarray(
        ids.reshape(B, 4, 128, 1).astype(np.float32)
    )
    woutT_full = np.ascontiguousarray(W_out.T.astype(BF))  # [D, V]

    in_maps = []
    for c in range(NCORES):
        b = c // 2
        t0 = (c % 2) * TC
        v0 = c * VS
        v1 = min(v0 + VS, V)
        realw = v1 - v0

        dec_my = decoder_output[b, t0 : t0 + TC]  # [TC, D]
        dec_myT = np.ascontiguousarray(dec_my.T)  # [D, TC]
        membT_h = np.ascontiguousarray(memory_output[b].T)  # [D, M]
        memb_h = np.ascontiguousarray(memory_output[b])  # [M, D]
        L = int(msl[b])
        mrow = np.where(np.arange(M) < L, 0.0, MASK_NEG).astype(np.float32)
        maskb_h = np.ascontiguousarray(np.broadcast_to(mrow, (TC, M)))

        wt = np.zeros((D, VP), dtype=BF)
        wt[:, :realw] = woutT_full[:, v0:v1]
        woutT_h = np.ascontiguousarray(
            wt.reshape(D // 128, 128, NVT, 128).transpose(2, 0, 1, 3)
        )  # [NVT, KD, 128, 128]
        bo_pad = np.full(VP, PAD_BIAS, np.float32)
        bo_pad[:realw] = b_out_a[v0:v1]
        bo_h = np.ascontiguousarray(bo_pad.reshape(NVT, 128).T)  # [128, NVT]

        loc = ids - v0  # [B, M]
        valid = (ids >= v0) & (ids < v1) & (np.arange(M)[None, :] < msl[:, None])
        loc = np.where(valid, loc, SENT).astype(np.int32)
        # dedup: only the first occurrence of a vocab id per batch does the
        # RMW fixup (the selection-matmul merge already sums the whole group);
        # later occurrences would double-add.
        for bb_ in range(B):
            seen_ = set()
            for m_ in range(M):
                lv = int(loc[bb_, m_])
                if lv != SENT:
                    if lv in seen_:
                        loc[bb_, m_] = SENT
                    else:
                        seen_.add(lv)
        ids_loc_h = np.ascontiguousarray(loc.reshape(B, 4, 128, 1))

        in_maps.append(
            {
                "dec_myT": dec_myT,
                "dec_myT_bf": dec_myT.astype(BF),
                "decT": decT_bf,
                "wcopy": W_copy,
                "wdecT": wdecT,
                "wgenT": wgenT,
                "bdec": bdec_h,
                "bgen": bgen_h,
                "membT": membT_h,
                "memb": memb_h,
                "maskb": maskb_h,
                "woutT": woutT_h,
                "bo": bo_h,
                "ids_f": ids_f_h,
                "ids_loc": ids_loc_h,
            }
        )

    nc = _get_nc()
    res = run_bass_kernel_spmd(nc, in_maps, core_ids=list(range(NCORES)))

    out_full = np.empty((V, B, T), np.float32)
    for c in range(NCORES):
        v0 = c * VS
        v1 = min(v0 + VS, V)
        realw = v1 - v0
        for b in range(B):
            out_full[v0:v1, b, :] = res.results[c][f"out{b}"][:realw, :]
    return np.ascontiguousarray(out_full.transpose(1, 2, 0))



# revision 10
# speedup vs baseline: 1.1076x; 1.1076x over previous
"""Trainium2 Bass kernel for nn_CopyMechanismMixin (copy-mechanism + vocab projection).

v2 design (token-partition, fp8, log-domain):
- Vocab-parallel across 8 cores (VP=6400 padded shard). Main logits matmul in
  fp8 e4m3 DoubleRow perf mode with tokens on PSUM partitions: lhsT = dec
  chunks (stationary across vocab stripes), rhs = W_out^T chunks (moving).
- Log-domain output: out[t,v] = logit + b_out[v] + c[t], c = ln(p_gen) - ln(S).
  No per-element Ln/mult pass; exp only feeds the softmax denominator S
  (activation accum_out gives per-token partial sums for free).
- Copy-attention prologue token-parallel (128 tokens/core, fp16 matmuls),
  cross-core: AllGather of copy probs (bf16) + ln(p_gen) column, AllReduce of
  S per 256-token round (4 rounds, pipelined with compute).
- Scatter fixup: each core recomputes logits for the <=256 vocab rows hit by
  its assigned (batch, m-chunk) pair, does exp/add-merged-copy/ln in
  vocab-partition layout, and returns them in a small side output; the host
  places those columns during unsharding. No indirect RMW on the big output.
"""

import numpy as np
import ml_dtypes

import concourse.bass as bass
import concourse.bacc as bacc
import concourse.mybir as mybir
import concourse.tile as tile
from concourse.bass_utils import run_bass_kernel_spmd
from concourse.masks import make_identity

F32 = mybir.dt.float32
F16 = mybir.dt.float16
BF16 = mybir.dt.bfloat16
F8 = mybir.dt.float8e4
I32 = mybir.dt.int32
NPF8 = ml_dtypes.float8_e4m3
NPBF = ml_dtypes.bfloat16
AF = mybir.ActivationFunctionType
ALU = mybir.AluOpType
PM = mybir.MatmulPerfMode

B, T, M, D, V = 4, 256, 512, 1024, 50257
NC = 8
VS = -(-V // NC)                    # 6283 real vocab per core
VP = 6400                           # padded shard width
CHW = [512] * 12 + [256]            # vocab chunk widths
CHO = [0]
for w in CHW:
    CHO.append(CHO[-1] + w)
STRIPES = [list(range(0, 6)), list(range(6, 12)), [12]]
NT = B * T                          # 1024 tokens
TC = 128                            # tokens per core (phase A)
KD = 8                              # 128-wide contraction chunks of D
KG = 4                              # fp8 DoubleRow groups (256 contraction)
NR = 4                              # rounds (2 token-blocks each)
MASK_NEG = -30000.0
PAD_BIAS = -30.0                    # bias for pad vocab cols -> exp ~ 1e-13
OSPL = [(0, 3200), (3200, 3200)]  # out store splits of VP


def build_kernel(has_bo: bool = False):
    nc = bacc.Bacc(
        "TRN2",
        target_bir_lowering=False,
        debug=False,
        enable_asserts=False,
        num_devices=NC,
    )
    RG = [list(range(NC))]
    # ---------------- I/O ----------------
    dec_myT16 = nc.dram_tensor("dec_myT16", [D, TC], F16, kind="ExternalInput")
    dec8 = nc.dram_tensor("dec8", [KG, 128, 2, NT], F8, kind="ExternalInput")
    d8f_d = nc.dram_tensor("d8f", [128, KG, 2, T], F8, kind="ExternalInput")
    wcopy16 = nc.dram_tensor("wcopy16", [D, D], F16, kind="ExternalInput")
    wdecT16 = nc.dram_tensor("wdecT16", [2 * D, D], F16, kind="ExternalInput")
    membT16 = nc.dram_tensor("membT16", [D, M], F16, kind="ExternalInput")
    memb16 = nc.dram_tensor("memb16", [M, D], F16, kind="ExternalInput")
    maskb = nc.dram_tensor("maskb", [TC, M], F32, kind="ExternalInput")
    bdec_r = nc.dram_tensor("bdec_r", [1, D], F32, kind="ExternalInput")
    wgen_r16 = nc.dram_tensor("wgen_r16", [1, D], F16, kind="ExternalInput")
    bgen_d = nc.dram_tensor("bgen_d", [128, 1], F32, kind="ExternalInput")
    wout8 = nc.dram_tensor("wout8", [KG, 128, 2, VP], F8, kind="ExternalInput")
    bo_r16 = nc.dram_tensor("bo_r16", [1, VP], F16, kind="ExternalInput")
    idf4_d = nc.dram_tensor("idf4", [128, 4], F32, kind="ExternalInput")
    idfm_d = nc.dram_tensor("idfm", [128, 2], F32, kind="ExternalInput")
    boid_d = nc.dram_tensor("boid", [128, 2], F32, kind="ExternalInput")
    selo_d = nc.dram_tensor("selo", [8, 2], F32, kind="ExternalInput")
    offs_d = nc.dram_tensor("offs", [128, 8], I32, kind="ExternalInput")
    w8i_d = nc.dram_tensor("w8i", [128, KG, 2, 2 * TC], F8, kind="ExternalInput")
    npad_d = nc.dram_tensor("npad", [128, 1], F32, kind="ExternalInput")
    outD = nc.dram_tensor("outD", [NT, VP], F32, kind="ExternalOutput")
    outF = nc.dram_tensor("outF", [2, 128, T], F32, kind="ExternalOutput")

    with tile.TileContext(nc) as tc:
        with (
            tc.tile_pool(name="const", bufs=1) as cn,
            tc.tile_pool(name="wro", bufs=2) as wro,
            tc.tile_pool(name="pha", bufs=1) as pa,
            tc.tile_pool(name="sm", bufs=2) as sm,
            tc.tile_pool(name="stash", bufs=4) as stp,
            tc.tile_pool(name="est", bufs=2) as estp,
            tc.tile_pool(name="outs", bufs=2) as outp,
            tc.tile_pool(name="fix", bufs=1) as fxp,
            tc.tile_pool(name="psB", bufs=6, space="PSUM") as psB,
            tc.tile_pool(name="psS", bufs=2, space="PSUM") as psS,
            tc.tile_pool(name="dram", bufs=1, space="DRAM") as dram,
        ):
            ident = cn.tile([128, 128], F32, tag="ident")
            make_identity(nc, ident[:])
            ones1f = cn.tile([1, 128], F32, tag="ones1f")
            nc.vector.memset(ones1f[:], 1.0)
            ones1h = cn.tile([1, 128], F16, tag="ones1h")
            nc.vector.memset(ones1h[:], 1.0)

            # ---- phase A input loads (sync queue, in need-order) ----
            dmt3 = cn.tile([128, KD, TC], F16, tag="dmt")
            nc.sync.dma_start(
                out=dmt3[:], in_=dec_myT16[:].rearrange("(a p) t -> p a t", p=128)
            )
            maskt = cn.tile([128, M], F32, tag="maskt")
            nc.sync.dma_start(out=maskt[:], in_=maskb[:])
            bgt = cn.tile([128, 1], F32, tag="bgt")
            nc.sync.dma_start(out=bgt[:], in_=bgen_d[:])

            # ---- broadcast consts via ones-matmul ----
            bdb = cn.tile([128, D], BF16, tag="bdb")
            for h in range(2):
                bdt = wro.tile([1, 512], F32, tag="bdt", name=f"bdt{h}")
                nc.sync.dma_start(out=bdt[:], in_=bdec_r[:, h * 512 : (h + 1) * 512])
                ps = psB.tile([128, 512], F32, space="PSUM", tag="pb", name=f"bd{h}")
                nc.tensor.matmul(
                    out=ps[:], lhsT=ones1f[:], rhs=bdt[:], start=True, stop=True
                )
                nc.scalar.copy(bdb[:, h * 512 : (h + 1) * 512], ps[:])
            wgb = cn.tile([128, D], F16, tag="wgb")
            for h in range(2):
                wgt = wro.tile([1, 512], F16, tag="wgt", name=f"wgt{h}")
                nc.sync.dma_start(
                    out=wgt[:], in_=wgen_r16[:, h * 512 : (h + 1) * 512]
                )
                ps = psB.tile([128, 512], F32, space="PSUM", tag="pb", name=f"wg{h}")
                nc.tensor.matmul(
                    out=ps[:], lhsT=ones1h[:], rhs=wgt[:], start=True, stop=True
                )
                nc.scalar.copy(wgb[:, h * 512 : (h + 1) * 512], ps[:])
            bob = cn.tile([128, VP], BF16, tag="bob", name="bob") if has_bo else None
            for ch in range(13 if has_bo else 0):
                o, w = CHO[ch], CHW[ch]
                bort = wro.tile([1, 512], F16, tag="bort", name=f"bor{ch}")
                nc.sync.dma_start(out=bort[:, :w], in_=bo_r16[:, o : o + w])
                ps = psB.tile([128, 512], F32, space="PSUM", tag="pb", name=f"bo{ch}")
                nc.tensor.matmul(
                    out=ps[:, :w], lhsT=ones1h[:], rhs=bort[:, :w],
                    start=True, stop=True,
                )
                nc.scalar.copy(bob[:, o : o + w], ps[:, :w])

            # ============ Phase A: copy-attention for my 128 tokens ============
            # dprojT[d, t] = sum_e W_copy[e, d] * dec_myT[e, t]
            psdp = [
                psB.tile([128, 512], F32, space="PSUM", tag="pb", name=f"dp{h}")
                for h in range(2)
            ]
            for ke in range(KD):
                wck = wro.tile([128, D], F16, tag="wck", name=f"wck{ke}")
                nc.sync.dma_start(out=wck[:], in_=wcopy16[ke * 128 : (ke + 1) * 128, :])
                for dc in range(KD):
                    nc.tensor.matmul(
                        out=psdp[dc // 4][:, (dc % 4) * 128 : (dc % 4 + 1) * 128],
                        lhsT=wck[:, dc * 128 : (dc + 1) * 128],
                        rhs=dmt3[:, ke, :],
                        start=(ke == 0), stop=(ke == KD - 1),
                        skip_group_check=True,
                    )
            dpT = pa.tile([128, KD * 128], F16, tag="dpT")
            for h in range(2):
                nc.scalar.copy(
                    dpT[:, h * 512 : (h + 1) * 512], psdp[h][:]
                )

            # scores[t, m] = sum_d dprojT[d, t] * membT[d, m]
            pssc = psB.tile([128, M], F32, space="PSUM", tag="pb", name="sc")
            for dc in range(KD):
                mTt = wro.tile([128, M], F16, tag="mTt", name=f"mTt{dc}")
                nc.sync.dma_start(
                    out=mTt[:], in_=membT16[dc * 128 : (dc + 1) * 128, :]
                )
                nc.tensor.matmul(
                    out=pssc[:],
                    lhsT=dpT[:, dc * 128 : (dc + 1) * 128],
                    rhs=mTt[:],
                    start=(dc == 0), stop=(dc == KD - 1),
                )
            nc.vector.tensor_tensor(
                out=pssc[:], in0=pssc[:], in1=maskt[:], op=ALU.add
            )
            mx = sm.tile([128, 1], F32, tag="mx")
            nc.vector.reduce_max(out=mx[:], in_=pssc[:], axis=mybir.AxisListType.X)
            nmx = sm.tile([128, 1], F32, tag="nmx")
            nc.vector.tensor_scalar_mul(nmx[:], mx[:], -1.0)
            esc = pa.tile([128, M], F32, tag="esc")
            nc.scalar.activation(
                out=esc[:], in_=pssc[:], func=AF.Exp, bias=nmx[:, :1]
            )
            sesum = sm.tile([128, 1], F32, tag="ses")
            nc.vector.reduce_sum(out=sesum[:], in_=esc[:], axis=mybir.AxisListType.X)
            rinv = sm.tile([128, 1], F32, tag="rinv")
            nc.vector.reciprocal(rinv[:], sesum[:])
            attn = esc
            nc.vector.tensor_scalar_mul(attn[:], esc[:], rinv[:, :1])

            # attnT (fp16) for attn_output matmul
            aT = pa.tile([128, 4 * 128], F16, tag="aT")
            for mc in range(4):
                tp = psS.tile([128, 128], F32, space="PSUM", tag="ps", name=f"at{mc}")
                nc.tensor.transpose(
                    out=tp[:], in_=attn[:, mc * 128 : (mc + 1) * 128], identity=ident[:]
                )
                nc.scalar.copy(aT[:, mc * 128 : (mc + 1) * 128], tp[:])

            # attn_outT[d, t] = sum_m memb[m, d] * attnT[m, t]
            aoT = pa.tile([128, KD * 128], F16, tag="aoT")
            for dc in range(KD):
                mbt = wro.tile([128, 4, 128], F16, tag="mbt", name=f"mbt{dc}")
                nc.sync.dma_start(
                    out=mbt[:],
                    in_=memb16[:, dc * 128 : (dc + 1) * 128].rearrange(
                        "(a p) d -> p a d", p=128
                    ),
                )
                ps = psS.tile([128, 128], F32, space="PSUM", tag="ps", name=f"ao{dc}")
                for mc in range(4):
                    nc.tensor.matmul(
                        out=ps[:],
                        lhsT=mbt[:, mc, :],
                        rhs=aT[:, mc * 128 : (mc + 1) * 128],
                        start=(mc == 0), stop=(mc == 3),
                    )
                nc.scalar.copy(aoT[:, dc * 128 : (dc + 1) * 128], ps[:])

            # dwa[t, d] = tanh(cat[t, :] @ W_dec^T[:, d] + b_dec)  (t on partitions)
            psth = [
                psB.tile([128, 512], F32, space="PSUM", tag="pb", name=f"th{h}")
                for h in range(2)
            ]
            for eg in range(2 * KD):
                wdk = wro.tile([128, D], F16, tag="wdk", name=f"wdk{eg}")
                nc.sync.dma_start(
                    out=wdk[:], in_=wdecT16[eg * 128 : (eg + 1) * 128, :]
                )
                lhsT = (
                    dmt3[:, eg, :]
                    if eg < KD
                    else aoT[:, (eg - KD) * 128 : (eg - KD + 1) * 128]
                )
                for h in range(2):
                    nc.tensor.matmul(
                        out=psth[h][:],
                        lhsT=lhsT,
                        rhs=wdk[:, h * 512 : (h + 1) * 512],
                        start=(eg == 0), stop=(eg == 2 * KD - 1),
                    )
            th = pa.tile([128, D], F16, tag="th")
            for h in range(2):
                nc.vector.tensor_tensor(
                    out=psth[h][:], in0=psth[h][:],
                    in1=bdb[:, h * 512 : (h + 1) * 512], op=ALU.add,
                )
                nc.scalar.activation(
                    out=th[:, h * 512 : (h + 1) * 512], in_=psth[h][:], func=AF.Tanh
                )
            # z[t] = sum_d dwa[t,d] * W_gen[d]; pg = sigmoid(z + b_gen)
            zp = pa.tile([128, D], F16, tag="zp")
            nc.vector.tensor_tensor(out=zp[:], in0=th[:], in1=wgb[:], op=ALU.mult)
            z = sm.tile([128, 1], F32, tag="z")
            nc.vector.reduce_sum(out=z[:], in_=zp[:], axis=mybir.AxisListType.X)
            pg = sm.tile([128, 1], F32, tag="pg")
            nc.scalar.activation(out=pg[:], in_=z[:], func=AF.Sigmoid, bias=bgt[:, :1])
            lnpg = sm.tile([128, 1], F32, tag="lnpg")
            nc.scalar.activation(out=lnpg[:], in_=pg[:], func=AF.Ln)
            ag2_in = dram.tile([TC, 1], F32)
            nc.sync.dma_start(out=ag2_in[:], in_=lnpg[:])

            # cp[t, m] = attn * (1 - pg); transpose to [m, t] bf16 and AllGather
            ompg = sm.tile([128, 1], F32, tag="ompg")
            nc.vector.tensor_scalar(
                out=ompg[:], in0=pg[:], scalar1=-1.0, scalar2=1.0,
                op0=ALU.mult, op1=ALU.add,
            )
            cp = attn
            nc.vector.tensor_scalar_mul(cp[:], attn[:], ompg[:, :1])
            cpT = pa.tile([128, 4, 128], BF16, tag="cpT")
            for mc in range(4):
                tp = psS.tile([128, 128], F32, space="PSUM", tag="ps", name=f"ct{mc}")
                nc.tensor.transpose(
                    out=tp[:], in_=cp[:, mc * 128 : (mc + 1) * 128], identity=ident[:]
                )
                nc.scalar.copy(cpT[:, mc, :], tp[:])
            ag1_in = dram.tile([M, TC], BF16)
            nc.sync.dma_start(
                out=ag1_in[:].rearrange("(a p) t -> p a t", p=128), in_=cpT[:]
            )
            ag1_out = dram.tile([NC * M, TC], BF16, addr_space="Shared")
            nc.gpsimd.collective_compute(
                "AllGather", ALU.bypass, replica_groups=RG,
                ins=[ag1_in[:].opt()], outs=[ag1_out[:].opt()],
            )
            ag2_out = dram.tile([NT, 1], F32, addr_space="Shared")
            nc.gpsimd.collective_compute(
                "AllGather", ALU.bypass, replica_groups=RG,
                ins=[ag2_in[:].opt()], outs=[ag2_out[:].opt()],
            )
            lnpga = cn.tile([128, NC], F32, tag="lnpga")
            nc.sync.dma_start(
                out=lnpga[:], in_=ag2_out[:].rearrange("(a p) o -> p (a o)", p=128)
            )

            # ---- phase B weight loads ----
            da = [cn.tile([128, 2, NT], F8, tag=f"da{kg}", name=f"da{kg}") for kg in range(KG)]
            for kg in range(KG):
                nc.sync.dma_start(out=da[kg][:], in_=dec8[kg])
            wo = [cn.tile([128, 2, VP], F8, tag=f"wo{kg}", name=f"wo{kg}") for kg in range(KG)]
            for kg in range(KG):
                nc.sync.dma_start(out=wo[kg][:], in_=wout8[kg])
            # fixup inputs (small)
            idf4 = fxp.tile([128, 4], F32, tag="idf4")
            nc.sync.dma_start(out=idf4[:], in_=idf4_d[:])
            idfm = fxp.tile([128, 2], F32, tag="idfm")
            nc.sync.dma_start(out=idfm[:], in_=idfm_d[:])
            boid = fxp.tile([128, 2], F32, tag="boid")
            nc.sync.dma_start(out=boid[:], in_=boid_d[:])
            selo = fxp.tile([128, 2], F32, tag="selo")
            nc.sync.dma_start(out=selo[0:8, :], in_=selo_d[:])
            offs = fxp.tile([128, 8], I32, tag="offs")
            nc.sync.dma_start(out=offs[:], in_=offs_d[:])
            w8i = fxp.tile([128, KG, 2, 2 * TC], F8, tag="w8i")
            nc.sync.dma_start(out=w8i[:], in_=w8i_d[:])
            d8f = fxp.tile([128, KG, 2, T], F8, tag="d8f")
            nc.sync.dma_start(out=d8f[:], in_=d8f_d[:])
            npadt = fxp.tile([128, 1], F32, tag="npadt")
            nc.sync.dma_start(out=npadt[:], in_=npad_d[:])

            # ============ Phase B: logits rounds ============
            ar_in = [dram.tile([2 * TC, 1], F32, name=f"ari{r}") for r in range(NR)]
            ar_out = [
                dram.tile([2 * TC, 1], F32, addr_space="Shared", name=f"aro{r}")
                for r in range(NR)
            ]
            stash = {}
            c2 = {}

            def emit_round(r):
                for bi in range(2):
                    g = 2 * r + bi
                    st = stp.tile([128, VP], BF16, tag="st", name=f"st{g}")
                    stash[g] = st
                    sacc = sm.tile([128, 16], F32, tag="sacc", name=f"sa{g}")
                    for stripe in STRIPES:
                        pss = [
                            psB.tile(
                                [128, 512], F32, space="PSUM", tag="pb",
                                name=f"L{g}_{ch}",
                            )
                            for ch in stripe
                        ]
                        for kg in range(KG):
                            lhsT = da[kg][:, :, g * 128 : (g + 1) * 128]
                            for k, ch in enumerate(stripe):
                                nc.tensor.matmul(
                                    out=pss[k][:, : CHW[ch]],
                                    lhsT=lhsT,
                                    rhs=wo[kg][:, :, CHO[ch] : CHO[ch + 1]],
                                    start=(kg == 0), stop=(kg == KG - 1),
                                    perf_mode=PM.DoubleRow,
                                )
                        for k, ch in enumerate(stripe):
                            o, w = CHO[ch], CHW[ch]
                            if has_bo:
                                nc.vector.tensor_tensor(
                                    out=st[:, o : o + w], in0=pss[k][:, :w],
                                    in1=bob[:, o : o + w], op=ALU.add,
                                )
                            else:
                                nc.vector.tensor_copy(
                                    out=st[:, o : o + w], in_=pss[k][:, :w]
                                )
                            et = estp.tile(
                                [128, 512], BF16, tag="est", name=f"e{g}_{ch}"
                            )
                            nc.scalar.activation(
                                out=et[:, :w],
                                in_=st[:, o : o + w] if has_bo else pss[k][:, :w],
                                func=AF.Exp,
                                accum_out=sacc[:, ch : ch + 1],
                            )
                    sblk = sm.tile([128, 1], F32, tag="sblk", name=f"sb{g}")
                    nc.vector.reduce_sum(
                        out=sblk[:], in_=sacc[:, :13], axis=mybir.AxisListType.X
                    )
                    if not has_bo:
                        nc.vector.tensor_tensor(
                            out=sblk[:], in0=sblk[:], in1=npadt[:], op=ALU.add
                        )
                    nc.sync.dma_start(
                        out=ar_in[r][bi * TC : (bi + 1) * TC, :], in_=sblk[:]
                    )
                nc.gpsimd.collective_compute(
                    "AllReduce", ALU.add, replica_groups=RG,
                    ins=[ar_in[r][:].opt()], outs=[ar_out[r][:].opt()],
                )

            def emit_finals(r):
                sS = sm.tile([128, 2], F32, tag="sS", name=f"sS{r}")
                nc.sync.dma_start(
                    out=sS[:], in_=ar_out[r][:].rearrange("(a p) o -> p (a o)", p=128)
                )
                lnS = sm.tile([128, 2], F32, tag="lnS", name=f"lnS{r}")
                nc.scalar.activation(out=lnS[:], in_=sS[:], func=AF.Ln)
                c2t = sm.tile([128, 2], F32, tag="c2", name=f"c2{r}", bufs=4)
                nc.vector.tensor_tensor(
                    out=c2t[:], in0=lnpga[:, 2 * r : 2 * r + 2], in1=lnS[:],
                    op=ALU.subtract,
                )
                c2[r] = c2t
                for bi in range(2):
                    g = 2 * r + bi
                    for j, (o, w) in enumerate(OSPL):
                        ot = outp.tile([128, w], F32, tag="ot", name=f"o{g}_{j}")
                        nc.gpsimd.tensor_scalar_add(
                            ot[:], stash[g][:, o : o + w], c2t[:, bi : bi + 1]
                        )
                        nc.sync.dma_start(
                            out=outD[g * 128 : (g + 1) * 128, o : o + w], in_=ot[:]
                        )

            emit_round(0)
            emit_round(1)
            emit_finals(0)
            emit_round(2)
            emit_finals(1)
            emit_round(3)
            emit_finals(2)
            emit_finals(3)

            # ============ Fixup: recompute rows hit by copy-scatter ============
            # c8T: c in row form via transpose, then select my batch's 2 blocks
            ctile = fxp.tile([128, 128], F32, tag="ctile")
            nc.vector.memset(ctile[:], 0.0)
            for r in range(NR):
                nc.vector.tensor_copy(out=ctile[:, 2 * r : 2 * r + 2], in_=c2[r][:])
            tpc = psS.tile([128, 128], F32, space="PSUM", tag="ps", name="tpc")
            nc.tensor.transpose(out=tpc[:], in_=ctile[:], identity=ident[:])
            c8T = fxp.tile([128, 128], F32, tag="c8T")
            nc.scalar.copy(c8T[0:8, :], tpc[0:8, :])
            crow = fxp.tile([1, 2 * TC], F32, tag="crow")
            for j in range(2):
                psr = psS.tile([128, 128], F32, space="PSUM", tag="ps", name=f"pr{j}")
                nc.tensor.matmul(
                    out=psr[0:1, :], lhsT=selo[0:8, j : j + 1], rhs=c8T[0:8, :],
                    start=True, stop=True,
                )
                nc.scalar.copy(crow[:, j * 128 : (j + 1) * 128], psr[0:1, :])
            psc = psB.tile([128, 512], F32, space="PSUM", tag="pb", name="psc")
            nc.tensor.matmul(
                out=psc[:, : 2 * TC], lhsT=ones1f[:], rhs=crow[:], start=True, stop=True
            )
            PSb = fxp.tile([128, 2 * TC], BF16, tag="PSb")
            nc.scalar.activation(out=PSb[:], in_=psc[:, : 2 * TC], func=AF.Exp)

            # my ids transposed (for merge selection)
            idT = fxp.tile([128, 2 * 128], F32, tag="idT")
            for mi in range(2):
                tpi = psS.tile([128, 128], F32, space="PSUM", tag="ps", name=f"ti{mi}")
                nc.tensor.transpose(
                    out=tpi[:], in_=idfm[:, mi : mi + 1].to_broadcast([128, 128]),
                    identity=ident[:],
                )
                nc.scalar.copy(idT[:, mi * 128 : (mi + 1) * 128], tpi[:])
            # gather my batch's cp^T blocks from the AllGather output
            cpT4 = fxp.tile([128, 4, 2, 128], BF16, tag="cpT4")
            for j in range(8):
                mj, half = j // 2, j % 2
                nc.gpsimd.indirect_dma_start(
                    out=cpT4[:, mj, half, :],
                    out_offset=None,
                    in_=ag1_out[:],
                    in_offset=bass.IndirectOffsetOnAxis(ap=offs[:, j : j + 1], axis=0),
                )
            # merged copy mass per (my vocab row, token)
            mg = fxp.tile([128, 2 * T], F32, tag="mg")
            for mi in range(2):
                psm = psB.tile([128, 512], F32, space="PSUM", tag="pb", name=f"m{mi}")
                for mj in range(4):
                    sel = fxp.tile(
                        [128, 128], BF16, tag="sel", name=f"s{mi}_{mj}", bufs=2
                    )
                    nc.vector.tensor_tensor(
                        out=sel[:],
                        in0=idf4[:, mj : mj + 1].to_broadcast([128, 128]),
                        in1=idT[:, mi * 128 : (mi + 1) * 128],
                        op=ALU.is_equal,
                    )
                    nc.tensor.matmul(
                        out=psm[:, :T],
                        lhsT=sel[:],
                        rhs=cpT4[:, mj],
                        start=(mj == 0), stop=(mj == 3),
                    )
                nc.scalar.copy(mg[:, mi * T : (mi + 1) * T], psm[:, :T])
            # recompute logits for my rows, combine, log
            fixv = fxp.tile([128, 2, T], F32, tag="fixv")
            for mi in range(2):
                psf = psB.tile([128, 512], F32, space="PSUM", tag="pb", name=f"f{mi}")
                for kg in range(KG):
                    nc.tensor.matmul(
                        out=psf[:, :T],
                        lhsT=w8i[:, kg, :, mi * 128 : (mi + 1) * 128],
                        rhs=d8f[:, kg],
                        start=(kg == 0), stop=(kg == KG - 1),
                        perf_mode=PM.DoubleRow,
                    )
                ef = fxp.tile([128, T], BF16, tag="ef", name=f"ef{mi}", bufs=1)
                nc.scalar.activation(
                    out=ef[:], in_=psf[:, :T], func=AF.Exp, bias=boid[:, mi : mi + 1]
                )
                g1 = fxp.tile([128, T], F32, tag="g1", name=f"g1{mi}", bufs=1)
                nc.vector.tensor_tensor(out=g1[:], in0=ef[:], in1=PSb[:], op=ALU.mult)
                g2 = fxp.tile([128, T], F32, tag="g2", name=f"g2{mi}", bufs=1)
                nc.vector.tensor_tensor(
                    out=g2[:], in0=g1[:], in1=mg[:, mi * T : (mi + 1) * T], op=ALU.add
                )
                nc.scalar.activation(out=fixv[:, mi, :], in_=g2[:], func=AF.Ln)
            nc.gpsimd.dma_start(
                out=outF[:].rearrange("a p t -> p a t"), in_=fixv[:]
            )
    nc.finalize()
    return nc


_NC_CACHE = {}


def _get_nc(has_bo):
    if has_bo not in _NC_CACHE:
        _NC_CACHE[has_bo] = build_kernel(has_bo)
    return _NC_CACHE[has_bo]


def kernel(
    decoder_output,
    memory_output,
    memory_sequence_length,
    memory_ids,
    W_copy,
    b_copy,
    W_dec,
    b_dec,
    W_gen,
    b_gen,
    W_out,
    b_out,
):
    decoder_output = np.asarray(decoder_output, dtype=np.float32)
    memory_output = np.asarray(memory_output, dtype=np.float32)
    msl = np.asarray(memory_sequence_length).astype(np.int64)
    ids = np.asarray(memory_ids).astype(np.int64)
    W_copy = np.asarray(W_copy, dtype=np.float32)
    W_dec = np.asarray(W_dec, dtype=np.float32)
    W_gen = np.asarray(W_gen, dtype=np.float32)
    b_dec_a = np.asarray(b_dec, dtype=np.float32)
    b_gen_a = np.asarray(b_gen, dtype=np.float32)
    W_out = np.asarray(W_out, dtype=np.float32)
    b_out_a = np.asarray(b_out, dtype=np.float32)
    # NOTE: b_copy drops out: it shifts scores by a per-token constant, which
    # softmax over the memory axis cancels exactly.

    # ---- shared host prep ----
    dec_flat = decoder_output.reshape(NT, D)
    decT = np.ascontiguousarray(dec_flat.T)                      # [D, NT]
    dec8_h = np.ascontiguousarray(
        decT.astype(NPF8).reshape(KG, 2, 128, NT).transpose(0, 2, 1, 3)
    )                                                            # [KG,128,2,NT]
    W8 = W_out.T.astype(NPF8)                                    # [D, V]
    wcopy16_h = np.ascontiguousarray(W_copy.astype(np.float16))
    wdecT16_h = np.ascontiguousarray(W_dec.T.astype(np.float16))  # [2D, D]
    wgen_r16_h = np.ascontiguousarray(W_gen.reshape(1, D).astype(np.float16))
    bdec_r_h = np.ascontiguousarray(b_dec_a.reshape(1, D))
    bgen_h = np.full((128, 1), float(b_gen_a.ravel()[0]), np.float32)

    # per-batch dedup: first occurrence of each vocab id owns the fixup column
    owner = np.zeros((B, M), dtype=bool)
    for b in range(B):
        _, first_idx = np.unique(ids[b], return_index=True)
        owner[b, first_idx] = True

    in_maps = []
    for c in range(NC):
        b = c // 2
        t0 = (c % 2) * TC
        v0 = c * VS
        realw = min(VS, V - v0)

        dec_myT16_h = np.ascontiguousarray(
            decoder_output[b, t0 : t0 + TC].T.astype(np.float16)
        )
        membT16_h = np.ascontiguousarray(memory_output[b].T.astype(np.float16))
        memb16_h = np.ascontiguousarray(memory_output[b].astype(np.float16))
        L = int(msl[b])
        mrow = np.where(np.arange(M) < L, 0.0, MASK_NEG).astype(np.float32)
        maskb_h = np.ascontiguousarray(np.broadcast_to(mrow, (TC, M)))

        wt8 = np.zeros((D, VP), dtype=NPF8)
        wt8[:, :realw] = W8[:, v0 : v0 + realw]
        wout8_h = np.ascontiguousarray(
            wt8.reshape(KG, 2, 128, VP).transpose(0, 2, 1, 3)
        )                                                        # [KG,128,2,VP]
        bo_pad = np.full(VP, PAD_BIAS, np.float32)
        bo_pad[:realw] = b_out_a[v0 : v0 + realw]
        bo_r16_h = np.ascontiguousarray(bo_pad.reshape(1, VP).astype(np.float16))

        # fixup host inputs: batch b, m-chunks mc0, mc0+1
        mc0 = (c % 2) * 2
        idb = ids[b]
        myids = idb[mc0 * 128 : (mc0 + 2) * 128]                 # [256]
        idf4_h = np.ascontiguousarray(idb.reshape(4, 128).T.astype(np.float32))
        idfm_h = np.ascontiguousarray(
            idb.reshape(4, 128)[mc0 : mc0 + 2].T.astype(np.float32)
        )
        boid_h = np.ascontiguousarray(
            b_out_a[myids].reshape(2, 128).T.astype(np.float32)
        )
        selo_h = np.zeros((8, 2), np.float32)
        selo_h[2 * b, 0] = 1.0
        selo_h[2 * b + 1, 1] = 1.0
        offs_h = np.empty((128, 8), np.int32)
        for j in range(8):
            mj, half = j // 2, j % 2
            offs_h[:, j] = (2 * b + half) * M + mj * 128 + np.arange(128)
        w8g = W8[:, myids]                                       # [D, 256]
        w8i_h = np.ascontiguousarray(
            w8g.reshape(KG, 2, 128, 2 * TC).transpose(2, 0, 1, 3)
        )                                                        # [128,KG,2,256]
        d8f_h = np.ascontiguousarray(
            dec8_h[:, :, :, b * T : (b + 1) * T].transpose(1, 0, 2, 3)
        )                                                        # [128,KG,2,T]

        in_maps.append(
            {
                "dec_myT16": dec_myT16_h,
                "dec8": dec8_h,
                "d8f": d8f_h,
                "wcopy16": wcopy16_h,
                "wdecT16": wdecT16_h,
                "membT16": membT16_h,
                "memb16": memb16_h,
                "maskb": maskb_h,
                "bdec_r": bdec_r_h,
                "wgen_r16": wgen_r16_h,
                "bgen_d": bgen_h,
                "wout8": wout8_h,
                "bo_r16": bo_r16_h,
                "idf4": idf4_h,
                "idfm": idfm_h,
                "boid": boid_h,
                "selo": selo_h,
                "offs": offs_h,
                "w8i": w8i_h,
                "npad": np.full((128, 1), -(VP - realw), np.float32),
                "outD": None,
                "outF": None,
            }
        )
    for m in in_maps:
        del m["outD"], m["outF"]

    nc = _get_nc(bool(np.any(b_out_a)))
    res = run_bass_kernel_spmd(nc, in_maps, core_ids=list(range(NC)))

    out_full = np.empty((B, T, V), np.float32)
    for c in range(NC):
        v0 = c * VS
        realw = min(VS, V - v0)
        out_full[:, :, v0 : v0 + realw] = (
            res.results[c]["outD"][:, :realw].reshape(B, T, realw)
        )
    # place fixed-up copy-target columns
    for c in range(NC):
        b = c // 2
        mc0 = (c % 2) * 2
        fx = res.results[c]["outF"]                              # [2, 128, T]
        for j in range(2):
            mc = mc0 + j
            mask = owner[b, mc * 128 : (mc + 1) * 128]
            if not mask.any():
                continue
            vids = ids[b, mc * 128 : (mc + 1) * 128][mask]
            out_full[b][:, vids] = fx[j][mask].T
    return out_full
